# revision 27
# baseline (speedup 1.0000x reference)
"""Trainium2 Bass kernel for nn_ChangepointDetector.

Model (B=32, S=2048, I=32, W=20, H=128):
  win[t]  = x[t:t+20, :] flattened                      (sliding windows)
  h       = win @ W_enc + b_enc                         (B, nwin=2029, 128)
  enc     = gelu(LayerNorm(h) * gamma + beta)
  z1      = gelu([enc[t], enc[t+20]] @ W1 + b1)         (t in [0, T=2008))
  z2      = gelu(z1 @ W2 + b2)
  p       = sigmoid(z2 @ W3 + b3)                       -> pad to (B, S)

Sharding: pure data parallel, 4 batches per core across 8 cores.

Device kernel design (per core, channels-on-partitions layout):
  * The host pre-builds X4 [128, 2045] per batch (4 shift-stacked copies
    of xT: X4[32j+i, s] = x[s+j, i]), so the device input DMA is a
    handful of big contiguous 2D descriptors.  The encoder contraction
    (K = 20*32 = 640) becomes 5 dense K=128 matmuls per window tile with
    the rhs being plain column-offset views into X4.
  * The encoder runs in split-precision fp16: x and W_enc are decomposed
    on the host into hi + lo fp16 halves (x = xh + xl exactly to ~22
    mantissa bits), and x.W is computed as xh.Wh + xl.Wh + xh.Wl - three
    1-cycle/column fp16 matmuls accumulated in fp32 PSUM, ~1.3x faster
    than native fp32 (4 cycles/column) at near-fp32 accuracy.
  * The comparator L1 (K=2H, the big layer) uses the same fp16 hi/lo
    trick: enc is produced as an fp16 pair (ACT gelu + DVE cast/sub),
    and L1 is 6 accumulating 1-cyc/col matmuls.  L2 (M=64) stays fp32
    but runs as 2-way column-packed concurrent pairs, and L3 (K=64, M=1)
    as a 4-way row+column-packed quad.
  * Engine/queue discipline (the big perf lever vs. the naive version):
    all input DMA posting lives on the Sync (x hi halves + consts) and
    GpSimd (x lo halves) queues; the Scalar queue carries ONLY gelus and
    the Vector queue ONLY DVE compute, so neither chain queues behind
    ~650ns descriptor posts.
  * LN stats: per-window sum of squares via ones-column matmul (partition
    reduction), 4-way column-packed, issued one batch behind the encoder.
    W_enc/b_enc are mean-centered over H on the host so the LN mean is
    exactly zero and needs no subtraction.
  * rstd via a table-free Newton rsqrt run on the GPSIMD engine over
    narrow [4, 512] row views (rows {0,32,64,96} carry the stats; gpsimd
    cost scales with per-core elements so the narrow view is ~3x cheaper
    than full-width, and it keeps the DVE FIFO clear).  The PSUM read
    (var scale + eps) stays on DVE (gpsimd has no PSUM port).  rstd then
    takes a small DRAM bounce so a partition-step-0 DMA can broadcast it
    across partitions; gamma rides the ACT gelu's per-partition scale.
  * Emission is software-pipelined: the stats->rstd->broadcast->gelu
    chain of batch b hides under batch b+1's encoder matmuls.
  * Device returns pre-sigmoid logits; sigmoid + b3 + padding + threshold
    run on the host (monotonic, so probs > 0.5 matches z3 + b3 > 0).

The reference's probs concentrate near 0.5, so the boolean output cannot
survive genuinely low-precision matmuls (bf16 ~1e-2, float32r ~2e-4 were
measured and rejected); the fp16 hi/lo split keeps full fp32-class
accuracy (zero boolean flips measured on the grading inputs).
"""

import os
import numpy as np

# ---------------------------------------------------------------- constants
B, S, I, W, H = 32, 2048, 32, 20, 128
NWIN = S - W + 1          # 2029
T = S - 2 * W             # 2008
NCORES = 8
NB = B // NCORES          # 4 batches per core
KT = (W * I) // 128       # 5 k-tiles of 128
TN = [512, 512, 512, NWIN - 3 * 512]   # encoder window tiles (last 493)
CN = [512, 512, 512, T - 3 * 512]      # comparator tiles (last 472)
X4_COLS = NWIN + 4 * (KT - 1)          # 2045 columns of X4 actually used
LN_EPS = 1e-5

# Newton rsqrt seed: least-squares linear fit of v**-0.5 on [0.4, 1.85]
# (relative-error weighted).  2 NR iterations afterwards reach ~1e-7.
_vs = np.linspace(0.40, 1.85, 4001)
_w = _vs ** -0.25
_SEED_B, _SEED_A = np.polyfit(_vs, _vs ** -0.5, 1, w=_w)
NR_ITERS = 2

_BUILT = {}


def _build_nc():
    """Build + compile the single-core Bass program (same on all 8 cores)."""
    import concourse.bass as bass
    import concourse.tile as tile
    from concourse import bacc, mybir

    f32 = mybir.dt.float32
    f16 = mybir.dt.float16
    AF = mybir.ActivationFunctionType
    OP = mybir.AluOpType

    nc = bacc.Bacc(
        "TRN2",
        target_bir_lowering=False,
        debug=False,
        enable_asserts=True,
        num_devices=NCORES,
    )

    # host-prebuilt 4-shift stacks of xT, hi/lo fp16 halves
    xh4 = nc.dram_tensor("xh4", [NB, 128, X4_COLS], f16, kind="ExternalInput").ap()
    xl4 = nc.dram_tensor("xl4", [NB, 128, X4_COLS], f16, kind="ExternalInput").ap()
    # packed f16 weights: kt 0-4 wench, 5-9 wencl, 10-11 w1h, 12-13 w1l
    wpack = nc.dram_tensor("wpack", [128, 14, 128], f16, kind="ExternalInput").ap()
    # packed f32 consts: cols 0-63 w2, 64 w3 (dup x2), 65-72 vecs
    vpack = nc.dram_tensor("vpack", [128, 73], f32, kind="ExternalInput").ap()
    out = nc.dram_tensor("out", [NB, 4, 512], f32, kind="ExternalOutput").ap()

    def srows(t, n=512):
        # rows {0,32,64,96} of a [128, n] tile as a [4, n] strided AP
        return t.rearrange("(a b) n -> a b n", b=32)[:, 0, 0:n]

    from contextlib import ExitStack

    with tile.TileContext(nc) as tc, ExitStack() as ctx:
        consts = ctx.enter_context(tc.tile_pool(name="consts", bufs=1))
        x4p = ctx.enter_context(tc.tile_pool(name="x4p", bufs=1))
        hp = ctx.enter_context(tc.tile_pool(name="hp", bufs=3))
        sqp = ctx.enter_context(tc.tile_pool(name="sqp", bufs=6))
        prep = ctx.enter_context(tc.tile_pool(name="prep", bufs=2))
        enchp = ctx.enter_context(tc.tile_pool(name="enchp", bufs=4))
        enclp = ctx.enter_context(tc.tile_pool(name="enclp", bufs=3))
        encfp = ctx.enter_context(tc.tile_pool(name="encfp", bufs=4))
        z1p = ctx.enter_context(tc.tile_pool(name="z1p", bufs=3))
        z2p = ctx.enter_context(tc.tile_pool(name="z2p", bufs=3))
        prp = ctx.enter_context(tc.tile_pool(name="prp", bufs=2))
        nrp = ctx.enter_context(tc.tile_pool(name="nrp", bufs=1))
        lgp = ctx.enter_context(tc.tile_pool(name="lgp", bufs=2))
        php = ctx.enter_context(tc.tile_pool(name="php", bufs=3, space="PSUM"))
        psp = ctx.enter_context(tc.tile_pool(name="psp", bufs=1, space="PSUM"))
        pz1p = ctx.enter_context(tc.tile_pool(name="pz1p", bufs=2, space="PSUM"))
        pz2p = ctx.enter_context(tc.tile_pool(name="pz2p", bufs=1, space="PSUM"))
        pz3p = ctx.enter_context(tc.tile_pool(name="pz3p", bufs=1, space="PSUM"))
        drp = ctx.enter_context(tc.tile_pool(name="drp", bufs=4, space="DRAM"))

        wp_sb = consts.tile([128, 14, 128], f16, tag="wpack")
        nc.sync.dma_start(out=wp_sb[:, :, :], in_=wpack)
        vp_sb = consts.tile([128, 73], f32, tag="vpack")
        ones_sb = consts.tile([128, 128], f32, tag="ones")
        nc.vector.memset(ones_sb[:, :], 1.0)

        whi_sb = wp_sb[:, 0:KT, :]
        wlo_sb = wp_sb[:, KT : 2 * KT, :]
        w1h_sb = wp_sb[:, 2 * KT : 2 * KT + 2, :]
        w1l_sb = wp_sb[:, 2 * KT + 2 : 2 * KT + 4, :]
        w2_sb = vp_sb[:, 0:64]
        w3_sb = vp_sb[:, 64:65]
        bc_col = vp_sb[:, 65:66]      # centered encoder bias
        gamma_col = vp_sb[:, 66:67]
        beta_col = vp_sb[:, 67:68]
        b1_col = vp_sb[:, 68:69]
        b2_col = vp_sb[:, 69:70]      # b2 duplicated on rows 0-63 / 64-127

        # PE warmup: matmuls on constant data (no DMA dependency) ramp the
        # HAM clock gate while the first input DMAs are in flight.
        pwarm = php.tile([128, 512], f32, tag="ph")
        for _ in range(10):
            nc.tensor.matmul(
                pwarm[:, 0:128], lhsT=ones_sb[:, :], rhs=ones_sb[:, :],
                start=True, stop=True,
            )

        hs, sqs, rds, enchs, encls, encfs_all, pres_all, pss = (
            {}, {}, {}, {}, {}, {}, {}, {},
        )

        # batch 0 is split into two overlapping column regions so its first
        # window tiles can start after roughly half of the input DMA; later
        # batches land as one descriptor each (posted up front, queues clear)
        B0REG = ((0, 1040), (1024, X4_COLS - 1024))

        def emit_x4(b):
            # Only Sync and Scalar are HWDGE queues on TRN2 (gpsimd posts
            # go through Q7 SWDGE and thrash its ucode library against the
            # NR tensor ops).  All x batches post on Sync in batch order:
            # the DMA engines drain descriptors in post order, so batch 0
            # completes first and the encoder can start early.
            eng = nc.sync
            if b == 0:
                hs_ = [
                    x4p.tile([128, w], f16, tag=f"x4h0{k}", name=f"x4h0{k}")
                    for k, (c0, w) in enumerate(B0REG)
                ]
                ls_ = [
                    x4p.tile([128, w], f16, tag=f"x4l0{k}", name=f"x4l0{k}")
                    for k, (c0, w) in enumerate(B0REG)
                ]
                for k, (c0, w) in enumerate(B0REG):
                    eng.dma_start(out=hs_[k][:, 0:w], in_=xh4[0, :, c0 : c0 + w])
                    eng.dma_start(out=ls_[k][:, 0:w], in_=xl4[0, :, c0 : c0 + w])
                return hs_, ls_
            ht = x4p.tile([128, X4_COLS], f16, tag="x4h", bufs=3, name=f"x4h{b}")
            lt = x4p.tile([128, X4_COLS], f16, tag="x4l", bufs=3, name=f"x4l{b}")
            eng.dma_start(out=ht[:, :], in_=xh4[b])
            eng.dma_start(out=lt[:, :], in_=xl4[b])
            return [ht], [lt]

        def vpost():
            nc.scalar.dma_start(out=vp_sb[:, :], in_=vpack)

        # window tile j -> (x4 region index, local column base) per batch
        def jreg(b, j):
            if b == 0:
                return ((0, 0), (0, 512), (1, 0), (1, 512))[j]
            return (0, 512 * j)

        def emit_enc(b, x4, inserts=()):
            x4h, x4l = x4
            # ---- encoder GEMM + square per window tile -----------------
            h = hp.tile([128, S], f32, tag="h")
            hs[b] = h
            nc.vector.memset(h[:, NWIN:S], 0.0)  # sq j3 reads full 512
            sqs[b] = []
            ins = dict(inserts)
            for j in range(4):
                n, t0 = TN[j], 512 * j
                reg, lc = jreg(b, j)
                ph = php.tile([128, 512], f32, tag="ph")
                terms = []
                for kt in range(KT):
                    terms.append((whi_sb[:, kt, :], x4h[reg], kt))
                for kt in range(KT):
                    terms.append((whi_sb[:, kt, :], x4l[reg], kt))
                for kt in range(KT):
                    terms.append((wlo_sb[:, kt, :], x4h[reg], kt))
                for i, (wt, xs, kt) in enumerate(terms):
                    nc.tensor.matmul(
                        ph[:, 0:n],
                        lhsT=wt,
                        rhs=xs[:, lc + 4 * kt : lc + 4 * kt + n],
                        start=(i == 0),
                        stop=(i == len(terms) - 1),
                    )
                    if (j, i) in ins:
                        ins[(j, i)]()
                nc.vector.tensor_scalar_add(
                    out=h[:, t0 : t0 + n], in0=ph[:, 0:n], scalar1=bc_col
                )
                sq = sqp.tile([128, 512], f32, tag="sq")
                sqs[b].append(sq)
                # full 512 cols (h zero-padded) so the stats rows are fully
                # initialized without a PSUM memset
                nc.vector.tensor_mul(
                    out=sq[:, 0:512], in0=h[:, t0 : t0 + 512],
                    in1=h[:, t0 : t0 + 512],
                )
                if j in ins:
                    ins[j]()

        def emit_stats(b):
            # 4-way column-packed burst of ones-matmul partition reductions.
            # The bank is memset first (cheap) so the full-width NR reads
            # below never see uninitialized PSUM rows; the matmuls then
            # overwrite rows {0,32,64,96}.
            ps = psp.tile([128, 512], f32, tag="ps")
            pss[b] = ps
            nc.vector.memset(ps[:, :], 0.0)
            for j in range(4):
                nc.tensor.matmul(
                    ps[32 * j : 32 * j + 1, 0:512],
                    lhsT=ones_sb[:, 0:1],
                    rhs=sqs[b][j][:, 0:512],
                    start=True,
                    stop=True,
                    tile_position=(0, 32 * j),
                )

        def emit_nr(b):
            ps = pss[b]
            # ---- rstd = (var + eps)**-0.5 via Newton ------------------
            # Engines cannot stride the partition dim, so the 4 real stats
            # rows are bounced through DRAM into a compact [128, 16] tile:
            # vsh[p, 4a+chi] = var(tile a, window col 128*chi + p).  The
            # Newton chain then runs on GPSIMD over 16 columns (all 8 Q7
            # cores, 256 elem each) instead of 512 full-width - ~25x less
            # work, off the DVE FIFO entirely.
            # v = ps/H + eps: the PSUM -> SBUF bounce (only DVE/ACT can read
            # PSUM).  It lives on DVE, where after moving norm_pre off to
            # gpsimd every queued op is promptly-ready (PE-gated), so the
            # strict FIFO can never head-of-line-block this chain starter.
            v = nrp.tile([128, 512], f32, tag="v")
            nc.vector.tensor_scalar(
                out=v[:, :], in0=ps[:, :], scalar1=1.0 / H, scalar2=LN_EPS,
                op0=OP.mult, op1=OP.add,
            )
            # DMA1: rows {0,32,64,96} -> DRAM as a flat [4, 512] image
            rdv = drp.tile([4, 512], f32, tag="rdv")
            nc.sync.dma_start(out=rdv[:, :], in_=srows(v))
            # DMA2: read the same 2048 values back as [128, 16] - a pure
            # linear reshape (both sides traverse the buffer in flat order)
            vsh = nrp.tile([128, 16], f32, tag="vsh")
            rdv_flat = bass.AP(
                tensor=rdv.tensor, offset=rdv.offset, ap=[[16, 128], [1, 16]]
            )
            nc.sync.dma_start(out=vsh[:, :], in_=rdv_flat)
            ya = nrp.tile([128, 16], f32, tag="ya")
            yb = nrp.tile([128, 16], f32, tag="yb")
            nc.gpsimd.tensor_scalar(
                out=ya[:, :], in0=vsh[:, :], scalar1=float(_SEED_B),
                scalar2=float(_SEED_A), op0=OP.mult, op1=OP.add,
            )
            ycur, ynxt = ya, yb
            for _ in range(NR_ITERS):
                y2 = nrp.tile([128, 16], f32, tag="y2")
                nc.gpsimd.tensor_mul(out=y2[:, :], in0=ycur[:, :], in1=ycur[:, :])
                nc.gpsimd.tensor_mul(out=y2[:, :], in0=y2[:, :], in1=vsh[:, :])
                nc.gpsimd.tensor_scalar(
                    out=y2[:, :], in0=y2[:, :], scalar1=-0.5, scalar2=1.5,
                    op0=OP.mult, op1=OP.add,
                )
                nc.gpsimd.tensor_mul(
                    out=ynxt[:, :], in0=ycur[:, :], in1=y2[:, :]
                )
                ycur, ynxt = ynxt, ycur
            # rstd -> DRAM (flat) for the partition broadcast read-back
            rd = drp.tile([128, 16], f32, tag="rd")
            rds[b] = rd
            nc.sync.dma_start(out=rd[:, :], in_=ycur[:, :])

        def emit_norm_pre(b):
            # normalize: pre = h * rstd_bcast, one whole-batch broadcast
            # DMA + per-j DVE multiplies (gamma/beta ride the gelu)
            h = hs[b]
            rd = rds[b]
            pr = prp.tile([128, S], f32, tag="pr")
            # partition-broadcast read of the flat rstd buffer, split into
            # 4 per-tile chunks so norm_pre(j) can start as soon as its own
            # chunk lands: pr[p, 512a + c] = rstd[a, c]
            for a in range(4):
                chunk = bass.AP(
                    tensor=rd.tensor, offset=rd.offset + 512 * a,
                    ap=[[0, 128], [1, 512]],
                )
                nc.sync.dma_start(out=pr[:, 512 * a : 512 * a + 512], in_=chunk)
            # normalize multiplies on GPSIMD: they are gated by the bcast
            # DMA (the longest-latency op in the kernel), and on the DVE's
            # strict FIFO they head-of-line-blocked promptly-ready drains
            # and squares whenever the static order put them first.  On
            # gpsimd they serialize only with the rstd chain they follow.
            pre = prep.tile([128, S], f32, tag="pre")
            pres_all[b] = pre
            for j in range(4):
                n, t0 = TN[j], 512 * j
                nc.gpsimd.tensor_mul(
                    out=pre[:, t0 : t0 + n], in0=h[:, t0 : t0 + n],
                    in1=pr[:, t0 : t0 + n],
                )

        def emit_norm_fin(b):
            # gelu -> enc f32 (ACT only; the f16 hi/lo pair is derived by
            # DVE cast+sub later, scheduled where it can't block anything
            # the PE is about to need)
            pre = pres_all[b]
            encf = encfp.tile([128, S], f32, tag="encf")
            encfs_all[b] = encf
            for j in range(4):
                n, t0 = TN[j], 512 * j
                # gamma rides the ACT per-partition scale; beta the bias
                nc.scalar.activation(
                    out=encf[:, t0 : t0 + n], in_=pre[:, t0 : t0 + n],
                    func=AF.Gelu, bias=beta_col, scale=gamma_col,
                )

        def emit_subs(b, tiles=(0, 1, 2, 3)):
            # enc f16 pair: ench = f16(encf); encl = f16(encf - ench)
            encf = encfs_all[b]
            if b not in enchs:
                enchs[b] = enchp.tile([128, S], f16, tag="ench", name="ench")
                encls[b] = enclp.tile([128, S], f16, tag="encl", name="encl")
            ench, encl = enchs[b], encls[b]
            for j in tiles:
                n, t0 = TN[j], 512 * j
                nc.vector.tensor_copy(
                    out=ench[:, t0 : t0 + n], in_=encf[:, t0 : t0 + n]
                )
                nc.vector.tensor_sub(
                    out=encl[:, t0 : t0 + n], in0=encf[:, t0 : t0 + n],
                    in1=ench[:, t0 : t0 + n],
                )

        def emit_cmp(b, do_subs=True, after_l1j0=None):
            # ---- comparator MLP ----------------------------------------
            # PE order L1j0 L1j1 L1j2 L2A L1j3 L3j0 L3j1 L2B L3j2 L3j3:
            # every packed fp32 matmul's operands are ready >1 L1-group
            # before it issues, so the col/row-group concurrency is never
            # dependency-serialized.
            if do_subs:
                emit_subs(b)
            ench, encl = enchs[b], encls[b]
            pz3 = pz3p.tile([128, 512], f32, tag="pz3")
            nc.vector.memset(pz3[:, :], 0.0)

            z1s = [None] * 4

            def l1(j):
                n, t0 = CN[j], 512 * j
                pz1 = pz1p.tile([128, 512], f32, tag="pz1")
                terms = [
                    (w1h_sb[:, 0, :], ench, t0),
                    (w1h_sb[:, 1, :], ench, t0 + W),
                    (w1l_sb[:, 0, :], ench, t0),
                    (w1l_sb[:, 1, :], ench, t0 + W),
                    (w1h_sb[:, 0, :], encl, t0),
                    (w1h_sb[:, 1, :], encl, t0 + W),
                ]
                for i, (wt, e, c0) in enumerate(terms):
                    nc.tensor.matmul(
                        pz1[:, 0:n], lhsT=wt, rhs=e[:, c0 : c0 + n],
                        start=(i == 0), stop=(i == len(terms) - 1),
                    )
                if j == 0 and after_l1j0 is not None:
                    after_l1j0()
                z1 = z1p.tile([128, 512], f32, tag="z1")
                z1s[j] = z1
                nc.scalar.activation(
                    out=z1[:, 0:n], in_=pz1[:, 0:n], func=AF.Gelu,
                    bias=b1_col, scale=1.0,
                )

            def l2(pair):
                # 2-way column-packed pair into one PSUM bank
                pz2 = pz2p.tile([128, 512], f32, tag="pz2")
                if pair[1] == 3:
                    nc.vector.memset(pz2[64:128, CN[3] : 512], 0.0)
                for k, j in enumerate(pair):
                    nc.tensor.matmul(
                        pz2[64 * k : 64 * k + 64, 0 : CN[j]],
                        lhsT=w2_sb[:, :], rhs=z1s[j][:, 0 : CN[j]],
                        start=True, stop=True,
                        tile_position=(0, 64 * k),
                    )
                z2 = z2p.tile([128, 512], f32, tag="z2")
                nc.scalar.activation(
                    out=z2[:, :], in_=pz2[:, :], func=AF.Gelu,
                    bias=b2_col, scale=1.0,
                )
                return z2

            def l3(j, z2):
                # row+column-packed quad member into the shared pz3 bank
                r0 = 64 * (j % 2)
                nc.tensor.matmul(
                    pz3[32 * j : 32 * j + 1, 0 : CN[j]],
                    lhsT=w3_sb[r0 : r0 + 64, 0:1],
                    rhs=z2[r0 : r0 + 64, 0 : CN[j]],
                    start=True, stop=True,
                    tile_position=(r0, 32 * j),
                )

            l1(0)
            l1(1)
            l1(2)
            z2a = l2((0, 1))
            l1(3)
            l3(0, z2a)
            l3(1, z2a)
            z2b = l2((2, 3))
            l3(2, z2b)
            l3(3, z2b)
            # PSUM -> SBUF logits copy rides the Scalar engine (ACT Copy)
            # so the wait-on-L3 doesn't block the DVE queue
            lg = lgp.tile([128, 512], f32, tag="lg")
            nc.scalar.activation(out=lg[:, :], in_=pz3[:, :], func=AF.Copy)
            # out-DMA posts on the scalar queue, right behind its producer,
            # keeping the sync queue free for the rstd chain DMAs
            nc.scalar.dma_start(out=out[b], in_=srows(lg))

        # ---- software-pipelined emission -------------------------------
        # PE stream: enc0, enc1[stats0 mid-j0], enc2[stats1], enc3
        # [stats2], cmp0[stats3 after L1j0], cmp1..cmp3.  Each stats burst
        # is injected a few matmuls into the NEXT batch's first window
        # tile (just enough cover for the last sq to land), so the rstd
        # chain starts as early as possible; the chain then drains on
        # gpsimd/sync while the PE streams on.  norm_fin gelus are emitted
        # as encoder inserts so the ACT queue order is [v(b), gelus(b-1)]
        # and the latency-critical v never queues behind stale gelus.
        # subs(0) is split around enc3's j-tiles so enc3's last drains
        # (which gate stats(3)) aren't pushed behind 4 tiles of cast/sub.
        def stats_nr(b):
            return lambda: (emit_stats(b), emit_nr(b))

        def nfin(b):
            return lambda: emit_norm_fin(b)

        def subs_t(b, tiles):
            return lambda: emit_subs(b, tiles)

        vpost()
        x4s = [emit_x4(b) for b in range(NB)]
        emit_enc(0, x4s[0])
        emit_enc(1, x4s[1], inserts={(0, 7): stats_nr(0)})
        emit_norm_pre(0)
        emit_enc(2, x4s[2], inserts={(0, 7): stats_nr(1), 0: nfin(0)})
        emit_norm_pre(1)
        emit_enc(
            3, x4s[3],
            inserts={
                (0, 7): stats_nr(2),
                0: nfin(1),
                1: subs_t(0, (0, 1)),
                2: subs_t(0, (2, 3)),
            },
        )
        emit_norm_pre(2)
        emit_cmp(0, do_subs=False, after_l1j0=stats_nr(3))
        emit_subs(1)
        emit_norm_fin(2)
        emit_cmp(1, do_subs=False)
        emit_subs(2)
        emit_norm_pre(3)
        emit_norm_fin(3)
        emit_cmp(2, do_subs=False)
        emit_subs(3)
        emit_cmp(3, do_subs=False)

    nc.compile()
    return nc


def _get_nc():
    if "nc" not in _BUILT:
        _BUILT["nc"] = _build_nc()
    return _BUILT["nc"]


def make_in_maps(x, W_enc, b_enc, gamma, beta, W1, b1, W2, b2, W3, b3):
    """Host-side prep: shard x, build the 4-shift X4 stacks, center the
    encoder weights, pack weights/vectors into two DMA-able blobs."""
    x = np.ascontiguousarray(np.asarray(x, np.float32))
    W_enc = np.asarray(W_enc, np.float32)
    b_enc = np.asarray(b_enc, np.float32)

    W_c = W_enc - W_enc.mean(axis=1, keepdims=True)
    b_c = b_enc - b_enc.mean()
    wct = W_c.reshape(KT, 128, 128).transpose(1, 0, 2)
    wench = wct.astype(np.float16)
    wencl = (wct - wench.astype(np.float32)).astype(np.float16)
    w1t = np.asarray(W1, np.float32).reshape(2, 128, 128).transpose(1, 0, 2)
    w1h = w1t.astype(np.float16)
    w1l = (w1t - w1h.astype(np.float32)).astype(np.float16)
    wpack = np.ascontiguousarray(
        np.concatenate([wench, wencl, w1h, w1l], axis=1)
    )  # [128, 14, 128] f16

    vpack = np.zeros((128, 73), np.float32)
    vpack[:, 0:64] = np.asarray(W2, np.float32)
    w3c = np.asarray(W3, np.float32).reshape(64)
    vpack[0:64, 64] = w3c
    vpack[64:128, 64] = w3c
    vpack[:, 65] = b_c
    vpack[:, 66] = np.asarray(gamma, np.float32)
    vpack[:, 67] = np.asarray(beta, np.float32)
    vpack[:, 68] = np.asarray(b1, np.float32)
    b2f = np.asarray(b2, np.float32)
    vpack[0:64, 69] = b2f
    vpack[64:128, 69] = b2f

    xT = x.transpose(0, 2, 1)  # [B, 32, S]
    xTh = xT.astype(np.float16)
    xTl = (xT - xTh.astype(np.float32)).astype(np.float16)
    # X4[b, 32j+i, s] = xT[b, i, j+s]
    x4h = np.empty((B, 128, X4_COLS), np.float16)
    x4l = np.empty((B, 128, X4_COLS), np.float16)
    for j in range(4):
        x4h[:, 32 * j : 32 * j + 32, :] = xTh[:, :, j : j + X4_COLS]
        x4l[:, 32 * j : 32 * j + 32, :] = xTl[:, :, j : j + X4_COLS]

    in_maps = []
    for c in range(NCORES):
        sl = slice(NB * c, NB * (c + 1))
        in_maps.append(
            dict(
                xh4=np.ascontiguousarray(x4h[sl]),
                xl4=np.ascontiguousarray(x4l[sl]),
                wpack=wpack,
                vpack=vpack,
            )
        )
    return in_maps


def assemble_output(core_outs, b3):
    """core_outs: list of 8 arrays [NB, 4, 512] of pre-b3 logits."""
    b3 = float(np.asarray(b3).reshape(-1)[0])
    logits = np.zeros((B, T), np.float32)
    for c, o in enumerate(core_outs):
        for bb in range(NB):
            row = []
            for j in range(4):
                row.append(o[bb, j, 0 : CN[j]])
            logits[NB * c + bb] = np.concatenate(row)
    z = (logits + b3).astype(np.float32)
    p = (1.0 / (1.0 + np.exp(-z.astype(np.float64)))).astype(np.float32)
    probs = np.zeros((B, S), np.float32)
    probs[:, W : W + T] = p
    return probs, probs > 0.5


def kernel(**inputs):
    from concourse.bass_utils import run_bass_kernel_spmd

    nc = _get_nc()
    in_maps = make_in_maps(**inputs)
    res = run_bass_kernel_spmd(nc, in_maps, core_ids=list(range(NCORES)))
    core_outs = [res.results[c]["out"] for c in range(NCORES)]
    return assemble_output(core_outs, inputs["b3"])


# revision 29
# speedup vs baseline: 1.2601x; 1.2601x over previous
"""Trainium2 Bass kernel for nn_ChangepointDetector.

Model (B=32, S=2048, I=32, W=20, H=128):
  win[t]  = x[t:t+20, :] flattened                      (sliding windows)
  h       = win @ W_enc + b_enc                         (B, nwin=2029, 128)
  enc     = gelu(LayerNorm(h) * gamma + beta)
  z1      = gelu([enc[t], enc[t+20]] @ W1 + b1)         (t in [0, T=2008))
  z2      = gelu(z1 @ W2 + b2)
  p       = sigmoid(z2 @ W3 + b3)                       -> pad to (B, S)

Sharding: pure data parallel, 4 batches per core across 8 cores.

Device kernel design (per core, channels-on-partitions layout):
  * The host pre-builds X4 [128, 2045] per batch (4 shift-stacked copies
    of xT: X4[32j+i, s] = x[s+j, i]), so the device input DMA is a
    handful of big contiguous 2D descriptors.  The encoder contraction
    (K = 20*32 = 640) becomes 5 dense K=128 matmuls per window tile with
    the rhs being plain column-offset views into X4.
  * The encoder runs in split-precision fp16: x and W_enc are decomposed
    on the host into hi + lo fp16 halves (x = xh + xl exactly to ~22
    mantissa bits), and x.W is computed as xh.Wh + xl.Wh + xh.Wl - three
    1-cycle/column fp16 matmuls accumulated in fp32 PSUM, ~1.3x faster
    than native fp32 (4 cycles/column) at near-fp32 accuracy.
  * The comparator L1 (K=2H, the big layer) uses the same fp16 hi/lo
    trick: enc is produced as an fp16 pair (ACT gelu + DVE cast/sub),
    and L1 is 6 accumulating 1-cyc/col matmuls.  L2 (M=64) stays fp32
    but runs as 2-way column-packed concurrent pairs, and L3 (K=64, M=1)
    as a 4-way row+column-packed quad.
  * Engine/queue discipline (the big perf lever vs. the naive version):
    all input DMA posting lives on the Sync (x hi halves + consts) and
    GpSimd (x lo halves) queues; the Scalar queue carries ONLY gelus and
    the Vector queue ONLY DVE compute, so neither chain queues behind
    ~650ns descriptor posts.
  * LN stats: per-window sum of squares via ones-column matmul (partition
    reduction), 4-way column-packed, issued one batch behind the encoder.
    W_enc/b_enc are mean-centered over H on the host so the LN mean is
    exactly zero and needs no subtraction.
  * rstd via a table-free Newton rsqrt run on the GPSIMD engine over
    narrow [4, 512] row views (rows {0,32,64,96} carry the stats; gpsimd
    cost scales with per-core elements so the narrow view is ~3x cheaper
    than full-width, and it keeps the DVE FIFO clear).  The PSUM read
    (var scale + eps) stays on DVE (gpsimd has no PSUM port).  rstd then
    takes a small DRAM bounce so a partition-step-0 DMA can broadcast it
    across partitions; gamma rides the ACT gelu's per-partition scale.
  * Emission is software-pipelined: the stats->rstd->broadcast->gelu
    chain of batch b hides under batch b+1's encoder matmuls.
  * Device returns pre-sigmoid logits; sigmoid + b3 + padding + threshold
    run on the host (monotonic, so probs > 0.5 matches z3 + b3 > 0).

The reference's probs concentrate near 0.5, so the boolean output cannot
survive genuinely low-precision matmuls (bf16 ~1e-2, float32r ~2e-4 were
measured and rejected); the fp16 hi/lo split keeps full fp32-class
accuracy (zero boolean flips measured on the grading inputs).
"""

import os
import numpy as np

# ---------------------------------------------------------------- constants
B, S, I, W, H = 32, 2048, 32, 20, 128
NWIN = S - W + 1          # 2029
T = S - 2 * W             # 2008
NCORES = 8
NB = B // NCORES          # 4 batches per core
KT = (W * I) // 128       # 5 k-tiles of 128
TN = [512, 512, 512, NWIN - 3 * 512]   # encoder window tiles (last 493)
CN = [512, 512, 512, T - 3 * 512]      # comparator tiles (last 472)
X4_COLS = NWIN + 4 * (KT - 1)          # 2045 columns of X4 actually used
LN_EPS = 1e-5

# Newton rsqrt seed: least-squares linear fit of v**-0.5 on [0.4, 1.85]
# (relative-error weighted).  2 NR iterations afterwards reach ~1e-7.
_vs = np.linspace(0.40, 1.85, 4001)
_w = _vs ** -0.25
_SEED_B, _SEED_A = np.polyfit(_vs, _vs ** -0.5, 1, w=_w)
NR_ITERS = 2

_BUILT = {}


def _build_nc():
    """Build + compile the single-core Bass program (same on all 8 cores)."""
    import concourse.bass as bass
    import concourse.tile as tile
    from concourse import bacc, mybir

    f32 = mybir.dt.float32
    f16 = mybir.dt.float16
    AF = mybir.ActivationFunctionType
    OP = mybir.AluOpType

    nc = bacc.Bacc(
        "TRN2",
        target_bir_lowering=False,
        debug=False,
        enable_asserts=True,
        num_devices=NCORES,
    )

    # host-prebuilt 4-shift stacks of xT, hi/lo fp16 halves
    xh4 = nc.dram_tensor("xh4", [NB, 128, X4_COLS], f16, kind="ExternalInput").ap()
    xl4 = nc.dram_tensor("xl4", [NB, 128, X4_COLS], f16, kind="ExternalInput").ap()
    # packed f16 weights: kt 0-4 wench, 5-9 wencl, 10-11 w1h, 12-13 w1l
    wpack = nc.dram_tensor("wpack", [128, 14, 128], f16, kind="ExternalInput").ap()
    # packed f32 consts: cols 0-63 w2, 64 w3 (dup x2), 65-72 vecs
    vpack = nc.dram_tensor("vpack", [128, 73], f32, kind="ExternalInput").ap()
    out = nc.dram_tensor("out", [NB, 4, 512], f32, kind="ExternalOutput").ap()

    def srows(t, n=512):
        # rows {0,32,64,96} of a [128, n] tile as a [4, n] strided AP
        return t.rearrange("(a b) n -> a b n", b=32)[:, 0, 0:n]

    from contextlib import ExitStack

    with tile.TileContext(nc) as tc, ExitStack() as ctx:
        consts = ctx.enter_context(tc.tile_pool(name="consts", bufs=1))
        x4p = ctx.enter_context(tc.tile_pool(name="x4p", bufs=1))
        hp = ctx.enter_context(tc.tile_pool(name="hp", bufs=3))
        sqp = ctx.enter_context(tc.tile_pool(name="sqp", bufs=6))
        prep = ctx.enter_context(tc.tile_pool(name="prep", bufs=2))
        enchp = ctx.enter_context(tc.tile_pool(name="enchp", bufs=4))
        enclp = ctx.enter_context(tc.tile_pool(name="enclp", bufs=3))
        encfp = ctx.enter_context(tc.tile_pool(name="encfp", bufs=4))
        z1p = ctx.enter_context(tc.tile_pool(name="z1p", bufs=3))
        z2p = ctx.enter_context(tc.tile_pool(name="z2p", bufs=3))
        prp = ctx.enter_context(tc.tile_pool(name="prp", bufs=2))
        nrp = ctx.enter_context(tc.tile_pool(name="nrp", bufs=1))
        lgp = ctx.enter_context(tc.tile_pool(name="lgp", bufs=2))
        php = ctx.enter_context(tc.tile_pool(name="php", bufs=3, space="PSUM"))
        psp = ctx.enter_context(tc.tile_pool(name="psp", bufs=1, space="PSUM"))
        pz1p = ctx.enter_context(tc.tile_pool(name="pz1p", bufs=2, space="PSUM"))
        pz2p = ctx.enter_context(tc.tile_pool(name="pz2p", bufs=1, space="PSUM"))
        pz3p = ctx.enter_context(tc.tile_pool(name="pz3p", bufs=1, space="PSUM"))
        drp = ctx.enter_context(tc.tile_pool(name="drp", bufs=4, space="DRAM"))

        wp_sb = consts.tile([128, 14, 128], f16, tag="wpack")
        nc.sync.dma_start(out=wp_sb[:, :, :], in_=wpack)
        vp_sb = consts.tile([128, 73], f32, tag="vpack")
        ones_sb = consts.tile([128, 128], f32, tag="ones")
        nc.vector.memset(ones_sb[:, :], 1.0)

        whi_sb = wp_sb[:, 0:KT, :]
        wlo_sb = wp_sb[:, KT : 2 * KT, :]
        w1h_sb = wp_sb[:, 2 * KT : 2 * KT + 2, :]
        w1l_sb = wp_sb[:, 2 * KT + 2 : 2 * KT + 4, :]
        w2_sb = vp_sb[:, 0:64]
        w3_sb = vp_sb[:, 64:65]
        bc_col = vp_sb[:, 65:66]      # centered encoder bias
        gamma_col = vp_sb[:, 66:67]
        beta_col = vp_sb[:, 67:68]
        b1_col = vp_sb[:, 68:69]
        b2_col = vp_sb[:, 69:70]      # b2 duplicated on rows 0-63 / 64-127

        # PE warmup: matmuls on constant data (no DMA dependency) ramp the
        # HAM clock gate while the first input DMAs are in flight.
        pwarm = php.tile([128, 512], f32, tag="ph")
        for _ in range(10):
            nc.tensor.matmul(
                pwarm[:, 0:128], lhsT=ones_sb[:, :], rhs=ones_sb[:, :],
                start=True, stop=True,
            )

        hs, sqs, rds, enchs, encls, encfs_all, pres_all, pss = (
            {}, {}, {}, {}, {}, {}, {}, {},
        )

        # batch 0 is split into two overlapping column regions so its first
        # window tiles can start after roughly half of the input DMA; later
        # batches land as one descriptor each (posted up front, queues clear)
        B0REG = ((0, 1040), (1024, X4_COLS - 1024))

        def emit_x4(b):
            # Only Sync and Scalar are HWDGE queues on TRN2 (gpsimd posts
            # go through Q7 SWDGE and thrash its ucode library against the
            # NR tensor ops).  All x batches post on Sync in batch order:
            # the DMA engines drain descriptors in post order, so batch 0
            # completes first and the encoder can start early.
            eng = nc.sync
            if b == 0:
                hs_ = [
                    x4p.tile([128, w], f16, tag=f"x4h0{k}", name=f"x4h0{k}")
                    for k, (c0, w) in enumerate(B0REG)
                ]
                ls_ = [
                    x4p.tile([128, w], f16, tag=f"x4l0{k}", name=f"x4l0{k}")
                    for k, (c0, w) in enumerate(B0REG)
                ]
                for k, (c0, w) in enumerate(B0REG):
                    eng.dma_start(out=hs_[k][:, 0:w], in_=xh4[0, :, c0 : c0 + w])
                    eng.dma_start(out=ls_[k][:, 0:w], in_=xl4[0, :, c0 : c0 + w])
                return hs_, ls_
            ht = x4p.tile([128, X4_COLS], f16, tag="x4h", bufs=3, name=f"x4h{b}")
            lt = x4p.tile([128, X4_COLS], f16, tag="x4l", bufs=3, name=f"x4l{b}")
            eng.dma_start(out=ht[:, :], in_=xh4[b])
            eng.dma_start(out=lt[:, :], in_=xl4[b])
            return [ht], [lt]

        def vpost():
            nc.scalar.dma_start(out=vp_sb[:, :], in_=vpack)

        # window tile j -> (x4 region index, local column base) per batch
        def jreg(b, j):
            if b == 0:
                return ((0, 0), (0, 512), (1, 0), (1, 512))[j]
            return (0, 512 * j)

        def emit_enc(b, x4, inserts=()):
            x4h, x4l = x4
            # ---- encoder GEMM + square per window tile -----------------
            h = hp.tile([128, S], f32, tag="h")
            hs[b] = h
            nc.vector.memset(h[:, NWIN:S], 0.0)  # sq j3 reads full 512
            sqs[b] = []
            ins = dict(inserts)
            for j in range(4):
                n, t0 = TN[j], 512 * j
                reg, lc = jreg(b, j)
                ph = php.tile([128, 512], f32, tag="ph")
                terms = []
                for kt in range(KT):
                    terms.append((whi_sb[:, kt, :], x4h[reg], kt))
                for kt in range(KT):
                    terms.append((whi_sb[:, kt, :], x4l[reg], kt))
                for kt in range(KT):
                    terms.append((wlo_sb[:, kt, :], x4h[reg], kt))
                for i, (wt, xs, kt) in enumerate(terms):
                    nc.tensor.matmul(
                        ph[:, 0:n],
                        lhsT=wt,
                        rhs=xs[:, lc + 4 * kt : lc + 4 * kt + n],
                        start=(i == 0),
                        stop=(i == len(terms) - 1),
                    )
                    if (j, i) in ins:
                        ins[(j, i)]()
                nc.vector.tensor_scalar_add(
                    out=h[:, t0 : t0 + n], in0=ph[:, 0:n], scalar1=bc_col
                )
                sq = sqp.tile([128, 512], f32, tag="sq")
                sqs[b].append(sq)
                # full 512 cols (h zero-padded) so the stats rows are fully
                # initialized without a PSUM memset
                nc.vector.tensor_mul(
                    out=sq[:, 0:512], in0=h[:, t0 : t0 + 512],
                    in1=h[:, t0 : t0 + 512],
                )
                if j in ins:
                    ins[j]()

        def emit_stats(b):
            # 4-way column-packed burst of ones-matmul partition reductions.
            # The bank is memset first (cheap) so the full-width NR reads
            # below never see uninitialized PSUM rows; the matmuls then
            # overwrite rows {0,32,64,96}.
            ps = psp.tile([128, 512], f32, tag="ps")
            pss[b] = ps
            nc.vector.memset(ps[:, :], 0.0)
            for j in range(4):
                nc.tensor.matmul(
                    ps[32 * j : 32 * j + 1, 0:512],
                    lhsT=ones_sb[:, 0:1],
                    rhs=sqs[b][j][:, 0:512],
                    start=True,
                    stop=True,
                    tile_position=(0, 32 * j),
                )

        def emit_nr(b):
            ps = pss[b]
            # ---- rstd = (var + eps)**-0.5 via Newton ------------------
            # Engines cannot stride the partition dim, so the 4 real stats
            # rows are bounced through DRAM into a compact [128, 16] tile:
            # vsh[p, 4a+chi] = var(tile a, window col 128*chi + p).  The
            # Newton chain then runs on GPSIMD over 16 columns (all 8 Q7
            # cores, 256 elem each) instead of 512 full-width - ~25x less
            # work, off the DVE FIFO entirely.
            # v = ps/H + eps on the Scalar engine: ACT reads PSUM, Copy is
            # exact (no spline table), and it keeps this latency-critical
            # step out of the DVE FIFO where it queued behind normalize work
            v = nrp.tile([128, 512], f32, tag="v")
            nc.scalar.activation(
                out=v[:, :], in_=ps[:, :], func=AF.Copy,
                bias=LN_EPS, scale=1.0 / H,
            )
            # DMA1: rows {0,32,64,96} -> DRAM as a flat [4, 512] image.
            # Posted on the scalar queue right behind its producer v, so
            # there is no cross-engine semaphore hop before the DMA starts.
            rdv = drp.tile([4, 512], f32, tag="rdv")
            nc.scalar.dma_start(out=rdv[:, :], in_=srows(v))
            # DMA2: read the same 2048 values back as [128, 16] - a pure
            # linear reshape (both sides traverse the buffer in flat order)
            vsh = nrp.tile([128, 16], f32, tag="vsh")
            rdv_flat = bass.AP(
                tensor=rdv.tensor, offset=rdv.offset, ap=[[16, 128], [1, 16]]
            )
            nc.sync.dma_start(out=vsh[:, :], in_=rdv_flat)
            ya = nrp.tile([128, 16], f32, tag="ya")
            yb = nrp.tile([128, 16], f32, tag="yb")
            nc.gpsimd.tensor_scalar(
                out=ya[:, :], in0=vsh[:, :], scalar1=float(_SEED_B),
                scalar2=float(_SEED_A), op0=OP.mult, op1=OP.add,
            )
            ycur, ynxt = ya, yb
            for _ in range(NR_ITERS):
                y2 = nrp.tile([128, 16], f32, tag="y2")
                nc.gpsimd.tensor_mul(out=y2[:, :], in0=ycur[:, :], in1=ycur[:, :])
                nc.gpsimd.tensor_mul(out=y2[:, :], in0=y2[:, :], in1=vsh[:, :])
                nc.gpsimd.tensor_scalar(
                    out=y2[:, :], in0=y2[:, :], scalar1=-0.5, scalar2=1.5,
                    op0=OP.mult, op1=OP.add,
                )
                nc.gpsimd.tensor_mul(
                    out=ynxt[:, :], in0=ycur[:, :], in1=y2[:, :]
                )
                ycur, ynxt = ynxt, ycur
            # rstd -> DRAM (flat) for the partition broadcast read-back
            rd = drp.tile([128, 16], f32, tag="rd")
            rds[b] = rd
            nc.sync.dma_start(out=rd[:, :], in_=ycur[:, :])

        def emit_norm_pre(b):
            # normalize: pre = h * rstd_bcast, one whole-batch broadcast
            # DMA + per-j DVE multiplies (gamma/beta ride the gelu)
            h = hs[b]
            rd = rds[b]
            pr = prp.tile([128, S], f32, tag="pr")
            # partition-broadcast read of the flat rstd buffer, split into
            # 4 per-tile chunks so norm_pre(j) can start as soon as its own
            # chunk lands: pr[p, 512a + c] = rstd[a, c]
            for a in range(4):
                chunk = bass.AP(
                    tensor=rd.tensor, offset=rd.offset + 512 * a,
                    ap=[[0, 128], [1, 512]],
                )
                nc.sync.dma_start(out=pr[:, 512 * a : 512 * a + 512], in_=chunk)
            pre = prep.tile([128, S], f32, tag="pre")
            pres_all[b] = pre
            for j in range(4):
                n, t0 = TN[j], 512 * j
                nc.vector.tensor_mul(
                    out=pre[:, t0 : t0 + n], in0=h[:, t0 : t0 + n],
                    in1=pr[:, t0 : t0 + n],
                )

        def emit_norm_fin(b):
            # gelu -> enc f32 (ACT only; the f16 hi/lo pair is derived by
            # DVE cast+sub later, scheduled where it can't block anything
            # the PE is about to need)
            pre = pres_all[b]
            encf = encfp.tile([128, S], f32, tag="encf")
            encfs_all[b] = encf
            for j in range(4):
                n, t0 = TN[j], 512 * j
                # gamma rides the ACT per-partition scale; beta the bias
                nc.scalar.activation(
                    out=encf[:, t0 : t0 + n], in_=pre[:, t0 : t0 + n],
                    func=AF.Gelu, bias=beta_col, scale=gamma_col,
                )

        def emit_subs(b, tiles=(0, 1, 2, 3)):
            # enc f16 pair: ench = f16(encf); encl = f16(encf - ench)
            encf = encfs_all[b]
            if b not in enchs:
                enchs[b] = enchp.tile([128, S], f16, tag="ench", name="ench")
                encls[b] = enclp.tile([128, S], f16, tag="encl", name="encl")
            ench, encl = enchs[b], encls[b]
            for j in tiles:
                n, t0 = TN[j], 512 * j
                nc.vector.tensor_copy(
                    out=ench[:, t0 : t0 + n], in_=encf[:, t0 : t0 + n]
                )
                nc.vector.tensor_sub(
                    out=encl[:, t0 : t0 + n], in0=encf[:, t0 : t0 + n],
                    in1=ench[:, t0 : t0 + n],
                )

        def emit_cmp(b, do_subs=True, after_l1j0=None):
            # ---- comparator MLP ----------------------------------------
            # PE order L1j0 L1j1 L1j2 L2A L1j3 L3j0 L3j1 L2B L3j2 L3j3:
            # every packed fp32 matmul's operands are ready >1 L1-group
            # before it issues, so the col/row-group concurrency is never
            # dependency-serialized.
            if do_subs:
                emit_subs(b)
            ench, encl = enchs[b], encls[b]
            pz3 = pz3p.tile([128, 512], f32, tag="pz3")
            nc.vector.memset(pz3[:, :], 0.0)

            z1s = [None] * 4

            def l1(j):
                n, t0 = CN[j], 512 * j
                pz1 = pz1p.tile([128, 512], f32, tag="pz1")
                terms = [
                    (w1h_sb[:, 0, :], ench, t0),
                    (w1h_sb[:, 1, :], ench, t0 + W),
                    (w1l_sb[:, 0, :], ench, t0),
                    (w1l_sb[:, 1, :], ench, t0 + W),
                    (w1h_sb[:, 0, :], encl, t0),
                    (w1h_sb[:, 1, :], encl, t0 + W),
                ]
                for i, (wt, e, c0) in enumerate(terms):
                    nc.tensor.matmul(
                        pz1[:, 0:n], lhsT=wt, rhs=e[:, c0 : c0 + n],
                        start=(i == 0), stop=(i == len(terms) - 1),
                    )
                if j == 0 and after_l1j0 is not None:
                    after_l1j0()
                z1 = z1p.tile([128, 512], f32, tag="z1")
                z1s[j] = z1
                nc.scalar.activation(
                    out=z1[:, 0:n], in_=pz1[:, 0:n], func=AF.Gelu,
                    bias=b1_col, scale=1.0,
                )

            def l2(pair):
                # 2-way column-packed pair into one PSUM bank
                pz2 = pz2p.tile([128, 512], f32, tag="pz2")
                if pair[1] == 3:
                    nc.vector.memset(pz2[64:128, CN[3] : 512], 0.0)
                for k, j in enumerate(pair):
                    nc.tensor.matmul(
                        pz2[64 * k : 64 * k + 64, 0 : CN[j]],
                        lhsT=w2_sb[:, :], rhs=z1s[j][:, 0 : CN[j]],
                        start=True, stop=True,
                        tile_position=(0, 64 * k),
                    )
                z2 = z2p.tile([128, 512], f32, tag="z2")
                nc.scalar.activation(
                    out=z2[:, :], in_=pz2[:, :], func=AF.Gelu,
                    bias=b2_col, scale=1.0,
                )
                return z2

            def l3(j, z2):
                # row+column-packed quad member into the shared pz3 bank
                r0 = 64 * (j % 2)
                nc.tensor.matmul(
                    pz3[32 * j : 32 * j + 1, 0 : CN[j]],
                    lhsT=w3_sb[r0 : r0 + 64, 0:1],
                    rhs=z2[r0 : r0 + 64, 0 : CN[j]],
                    start=True, stop=True,
                    tile_position=(r0, 32 * j),
                )

            l1(0)
            l1(1)
            l1(2)
            z2a = l2((0, 1))
            l1(3)
            l3(0, z2a)
            l3(1, z2a)
            z2b = l2((2, 3))
            l3(2, z2b)
            l3(3, z2b)
            # PSUM -> SBUF logits copy rides the Scalar engine (ACT Copy)
            # so the wait-on-L3 doesn't block the DVE queue
            lg = lgp.tile([128, 512], f32, tag="lg")
            nc.scalar.activation(out=lg[:, :], in_=pz3[:, :], func=AF.Copy)
            # out-DMA posts on the scalar queue, right behind its producer,
            # keeping the sync queue free for the rstd chain DMAs
            nc.scalar.dma_start(out=out[b], in_=srows(lg))

        # ---- software-pipelined emission -------------------------------
        # PE stream: enc0, enc1[stats0 mid-j0], enc2[stats1], enc3
        # [stats2], cmp0[stats3 after L1j0], cmp1..cmp3.  Each stats burst
        # is injected a few matmuls into the NEXT batch's first window
        # tile (just enough cover for the last sq to land), so the rstd
        # chain starts as early as possible; the chain then drains on
        # gpsimd/sync while the PE streams on.  norm_fin gelus are emitted
        # as encoder inserts so the ACT queue order is [v(b), gelus(b-1)]
        # and the latency-critical v never queues behind stale gelus.
        # subs(0) is split around enc3's j-tiles so enc3's last drains
        # (which gate stats(3)) aren't pushed behind 4 tiles of cast/sub.
        def stats_nr(b):
            return lambda: (emit_stats(b), emit_nr(b))

        def nfin(b):
            return lambda: emit_norm_fin(b)

        def subs_t(b, tiles):
            return lambda: emit_subs(b, tiles)

        vpost()
        x4s = [emit_x4(b) for b in range(NB)]
        emit_enc(0, x4s[0])
        emit_enc(1, x4s[1], inserts={(0, 7): stats_nr(0)})
        emit_norm_pre(0)
        emit_enc(2, x4s[2], inserts={(0, 7): stats_nr(1), 0: nfin(0)})
        emit_norm_pre(1)
        emit_enc(
            3, x4s[3],
            inserts={
                (0, 7): stats_nr(2),
                0: nfin(1),
                1: subs_t(0, (0, 1)),
                2: subs_t(0, (2, 3)),
            },
        )
        emit_norm_pre(2)
        emit_cmp(0, do_subs=False, after_l1j0=stats_nr(3))
        emit_subs(1)
        emit_norm_fin(2)
        emit_cmp(1, do_subs=False)
        emit_subs(2)
        emit_norm_pre(3)
        emit_norm_fin(3)
        emit_cmp(2, do_subs=False)
        emit_subs(3)
        emit_cmp(3, do_subs=False)

    nc.compile()
    return nc


def _get_nc():
    if "nc" not in _BUILT:
        _BUILT["nc"] = _build_nc()
    return _BUILT["nc"]


def make_in_maps(x, W_enc, b_enc, gamma, beta, W1, b1, W2, b2, W3, b3):
    """Host-side prep: shard x, build the 4-shift X4 stacks, center the
    encoder weights, pack weights/vectors into two DMA-able blobs."""
    x = np.ascontiguousarray(np.asarray(x, np.float32))
    W_enc = np.asarray(W_enc, np.float32)
    b_enc = np.asarray(b_enc, np.float32)

    W_c = W_enc - W_enc.mean(axis=1, keepdims=True)
    b_c = b_enc - b_enc.mean()
    wct = W_c.reshape(KT, 128, 128).transpose(1, 0, 2)
    wench = wct.astype(np.float16)
    wencl = (wct - wench.astype(np.float32)).astype(np.float16)
    w1t = np.asarray(W1, np.float32).reshape(2, 128, 128).transpose(1, 0, 2)
    w1h = w1t.astype(np.float16)
    w1l = (w1t - w1h.astype(np.float32)).astype(np.float16)
    wpack = np.ascontiguousarray(
        np.concatenate([wench, wencl, w1h, w1l], axis=1)
    )  # [128, 14, 128] f16

    vpack = np.zeros((128, 73), np.float32)
    vpack[:, 0:64] = np.asarray(W2, np.float32)
    w3c = np.asarray(W3, np.float32).reshape(64)
    vpack[0:64, 64] = w3c
    vpack[64:128, 64] = w3c
    vpack[:, 65] = b_c
    vpack[:, 66] = np.asarray(gamma, np.float32)
    vpack[:, 67] = np.asarray(beta, np.float32)
    vpack[:, 68] = np.asarray(b1, np.float32)
    b2f = np.asarray(b2, np.float32)
    vpack[0:64, 69] = b2f
    vpack[64:128, 69] = b2f

    xT = x.transpose(0, 2, 1)  # [B, 32, S]
    xTh = xT.astype(np.float16)
    xTl = (xT - xTh.astype(np.float32)).astype(np.float16)
    # X4[b, 32j+i, s] = xT[b, i, j+s]
    x4h = np.empty((B, 128, X4_COLS), np.float16)
    x4l = np.empty((B, 128, X4_COLS), np.float16)
    for j in range(4):
        x4h[:, 32 * j : 32 * j + 32, :] = xTh[:, :, j : j + X4_COLS]
        x4l[:, 32 * j : 32 * j + 32, :] = xTl[:, :, j : j + X4_COLS]

    in_maps = []
    for c in range(NCORES):
        sl = slice(NB * c, NB * (c + 1))
        in_maps.append(
            dict(
                xh4=np.ascontiguousarray(x4h[sl]),
                xl4=np.ascontiguousarray(x4l[sl]),
                wpack=wpack,
                vpack=vpack,
            )
        )
    return in_maps


def assemble_output(core_outs, b3):
    """core_outs: list of 8 arrays [NB, 4, 512] of pre-b3 logits."""
    b3 = float(np.asarray(b3).reshape(-1)[0])
    logits = np.zeros((B, T), np.float32)
    for c, o in enumerate(core_outs):
        for bb in range(NB):
            row = []
            for j in range(4):
                row.append(o[bb, j, 0 : CN[j]])
            logits[NB * c + bb] = np.concatenate(row)
    z = (logits + b3).astype(np.float32)
    p = (1.0 / (1.0 + np.exp(-z.astype(np.float64)))).astype(np.float32)
    probs = np.zeros((B, S), np.float32)
    probs[:, W : W + T] = p
    return probs, probs > 0.5


def kernel(**inputs):
    from concourse.bass_utils import run_bass_kernel_spmd

    nc = _get_nc()
    in_maps = make_in_maps(**inputs)
    res = run_bass_kernel_spmd(nc, in_maps, core_ids=list(range(NCORES)))
    core_outs = [res.results[c]["out"] for c in range(NCORES)]
    return assemble_output(core_outs, inputs["b3"])


# revision 31
# speedup vs baseline: 1.2661x; 1.0048x over previous
"""Trainium2 Bass kernel for nn_ChangepointDetector.

Model (B=32, S=2048, I=32, W=20, H=128):
  win[t]  = x[t:t+20, :] flattened                      (sliding windows)
  h       = win @ W_enc + b_enc                         (B, nwin=2029, 128)
  enc     = gelu(LayerNorm(h) * gamma + beta)
  z1      = gelu([enc[t], enc[t+20]] @ W1 + b1)         (t in [0, T=2008))
  z2      = gelu(z1 @ W2 + b2)
  p       = sigmoid(z2 @ W3 + b3)                       -> pad to (B, S)

Sharding: pure data parallel, 4 batches per core across 8 cores.

Device kernel design (per core, channels-on-partitions layout):
  * The host pre-builds X4 [128, 2045] per batch (4 shift-stacked copies
    of xT: X4[32j+i, s] = x[s+j, i]), so the device input DMA is a
    handful of big contiguous 2D descriptors.  The encoder contraction
    (K = 20*32 = 640) becomes 5 dense K=128 matmuls per window tile with
    the rhs being plain column-offset views into X4.
  * The encoder runs in split-precision fp16: x and W_enc are decomposed
    on the host into hi + lo fp16 halves (x = xh + xl exactly to ~22
    mantissa bits), and x.W is computed as xh.Wh + xl.Wh + xh.Wl - three
    1-cycle/column fp16 matmuls accumulated in fp32 PSUM, ~1.3x faster
    than native fp32 (4 cycles/column) at near-fp32 accuracy.
  * The comparator L1 (K=2H, the big layer) uses the same fp16 hi/lo
    trick: enc is produced as an fp16 pair (ACT gelu + DVE cast/sub),
    and L1 is 6 accumulating 1-cyc/col matmuls.  L2 (M=64) stays fp32
    but runs as 2-way column-packed concurrent pairs, and L3 (K=64, M=1)
    as a 4-way row+column-packed quad.
  * Engine/queue discipline (the big perf lever vs. the naive version):
    all input DMA posting lives on the Sync (x hi halves + consts) and
    GpSimd (x lo halves) queues; the Scalar queue carries ONLY gelus and
    the Vector queue ONLY DVE compute, so neither chain queues behind
    ~650ns descriptor posts.
  * LN stats: per-window sum of squares via ones-column matmul (partition
    reduction), 4-way column-packed, issued one batch behind the encoder.
    W_enc/b_enc are mean-centered over H on the host so the LN mean is
    exactly zero and needs no subtraction.
  * rstd via a table-free Newton rsqrt run on the GPSIMD engine over
    narrow [4, 512] row views (rows {0,32,64,96} carry the stats; gpsimd
    cost scales with per-core elements so the narrow view is ~3x cheaper
    than full-width, and it keeps the DVE FIFO clear).  The PSUM read
    (var scale + eps) stays on DVE (gpsimd has no PSUM port).  rstd then
    takes a small DRAM bounce so a partition-step-0 DMA can broadcast it
    across partitions; gamma rides the ACT gelu's per-partition scale.
  * Emission is software-pipelined: the stats->rstd->broadcast->gelu
    chain of batch b hides under batch b+1's encoder matmuls.
  * Device returns pre-sigmoid logits; sigmoid + b3 + padding + threshold
    run on the host (monotonic, so probs > 0.5 matches z3 + b3 > 0).

The reference's probs concentrate near 0.5, so the boolean output cannot
survive genuinely low-precision matmuls (bf16 ~1e-2, float32r ~2e-4 were
measured and rejected); the fp16 hi/lo split keeps full fp32-class
accuracy (zero boolean flips measured on the grading inputs).
"""

import os
import numpy as np

# ---------------------------------------------------------------- constants
B, S, I, W, H = 32, 2048, 32, 20, 128
NWIN = S - W + 1          # 2029
T = S - 2 * W             # 2008
NCORES = 8
NB = B // NCORES          # 4 batches per core
KT = (W * I) // 128       # 5 k-tiles of 128
TN = [512, 512, 512, NWIN - 3 * 512]   # encoder window tiles (last 493)
CN = [512, 512, 512, T - 3 * 512]      # comparator tiles (last 472)
X4_COLS = NWIN + 4 * (KT - 1)          # 2045 columns of X4 actually used
LN_EPS = 1e-5

# Newton rsqrt seed: least-squares linear fit of v**-0.5 on [0.4, 1.85]
# (relative-error weighted).  2 NR iterations afterwards reach ~1e-7.
_vs = np.linspace(0.40, 1.85, 4001)
_w = _vs ** -0.25
_SEED_B, _SEED_A = np.polyfit(_vs, _vs ** -0.5, 1, w=_w)
NR_ITERS = 2

_BUILT = {}


def _build_nc():
    """Build + compile the single-core Bass program (same on all 8 cores)."""
    import concourse.bass as bass
    import concourse.tile as tile
    from concourse import bacc, mybir

    f32 = mybir.dt.float32
    f16 = mybir.dt.float16
    AF = mybir.ActivationFunctionType
    OP = mybir.AluOpType

    nc = bacc.Bacc(
        "TRN2",
        target_bir_lowering=False,
        debug=False,
        enable_asserts=True,
        num_devices=NCORES,
    )

    # host-prebuilt 4-shift stacks of xT, hi/lo fp16 halves
    xh4 = nc.dram_tensor("xh4", [NB, 128, X4_COLS], f16, kind="ExternalInput").ap()
    xl4 = nc.dram_tensor("xl4", [NB, 128, X4_COLS], f16, kind="ExternalInput").ap()
    # packed f16 weights: kt 0-4 wench, 5-9 wencl, 10-11 w1h, 12-13 w1l
    wpack = nc.dram_tensor("wpack", [128, 14, 128], f16, kind="ExternalInput").ap()
    # packed f32 consts: cols 0-63 w2, 64 w3 (dup x2), 65-72 vecs
    vpack = nc.dram_tensor("vpack", [128, 73], f32, kind="ExternalInput").ap()
    out = nc.dram_tensor("out", [NB, 4, 512], f32, kind="ExternalOutput").ap()

    def srows(t, n=512):
        # rows {0,32,64,96} of a [128, n] tile as a [4, n] strided AP
        return t.rearrange("(a b) n -> a b n", b=32)[:, 0, 0:n]

    from contextlib import ExitStack

    with tile.TileContext(nc) as tc, ExitStack() as ctx:
        consts = ctx.enter_context(tc.tile_pool(name="consts", bufs=1))
        x4p = ctx.enter_context(tc.tile_pool(name="x4p", bufs=1))
        hp = ctx.enter_context(tc.tile_pool(name="hp", bufs=3))
        sqp = ctx.enter_context(tc.tile_pool(name="sqp", bufs=6))
        prep = ctx.enter_context(tc.tile_pool(name="prep", bufs=2))
        enchp = ctx.enter_context(tc.tile_pool(name="enchp", bufs=4))
        enclp = ctx.enter_context(tc.tile_pool(name="enclp", bufs=3))
        encfp = ctx.enter_context(tc.tile_pool(name="encfp", bufs=4))
        z1p = ctx.enter_context(tc.tile_pool(name="z1p", bufs=3))
        z2p = ctx.enter_context(tc.tile_pool(name="z2p", bufs=3))
        prp = ctx.enter_context(tc.tile_pool(name="prp", bufs=2))
        nrp = ctx.enter_context(tc.tile_pool(name="nrp", bufs=1))
        lgp = ctx.enter_context(tc.tile_pool(name="lgp", bufs=2))
        php = ctx.enter_context(tc.tile_pool(name="php", bufs=3, space="PSUM"))
        psp = ctx.enter_context(tc.tile_pool(name="psp", bufs=1, space="PSUM"))
        pz1p = ctx.enter_context(tc.tile_pool(name="pz1p", bufs=2, space="PSUM"))
        pz2p = ctx.enter_context(tc.tile_pool(name="pz2p", bufs=1, space="PSUM"))
        pz3p = ctx.enter_context(tc.tile_pool(name="pz3p", bufs=1, space="PSUM"))
        drp = ctx.enter_context(tc.tile_pool(name="drp", bufs=4, space="DRAM"))

        wp_sb = consts.tile([128, 14, 128], f16, tag="wpack")
        nc.sync.dma_start(out=wp_sb[:, :, :], in_=wpack)
        vp_sb = consts.tile([128, 73], f32, tag="vpack")
        ones_sb = consts.tile([128, 128], f32, tag="ones")
        nc.vector.memset(ones_sb[:, :], 1.0)

        whi_sb = wp_sb[:, 0:KT, :]
        wlo_sb = wp_sb[:, KT : 2 * KT, :]
        w1h_sb = wp_sb[:, 2 * KT : 2 * KT + 2, :]
        w1l_sb = wp_sb[:, 2 * KT + 2 : 2 * KT + 4, :]
        w2_sb = vp_sb[:, 0:64]
        w3_sb = vp_sb[:, 64:65]
        bc_col = vp_sb[:, 65:66]      # centered encoder bias
        gamma_col = vp_sb[:, 66:67]
        beta_col = vp_sb[:, 67:68]
        b1_col = vp_sb[:, 68:69]
        b2_col = vp_sb[:, 69:70]      # b2 duplicated on rows 0-63 / 64-127

        # PE warmup: matmuls on constant data (no DMA dependency) ramp the
        # HAM clock gate while the first input DMAs are in flight.
        pwarm = php.tile([128, 512], f32, tag="ph")
        for _ in range(10):
            nc.tensor.matmul(
                pwarm[:, 0:128], lhsT=ones_sb[:, :], rhs=ones_sb[:, :],
                start=True, stop=True,
            )

        hs, sqs, rds, enchs, encls, encfs_all, pres_all, pss = (
            {}, {}, {}, {}, {}, {}, {}, {},
        )

        # batch 0 is split into two overlapping column regions so its first
        # window tiles can start after roughly half of the input DMA; later
        # batches land as one descriptor each (posted up front, queues clear)
        B0REG = ((0, 1040), (1024, X4_COLS - 1024))

        def emit_x4(b):
            # Only Sync and Scalar are HWDGE queues on TRN2 (gpsimd posts
            # go through Q7 SWDGE and thrash its ucode library against the
            # NR tensor ops).  All x batches post on Sync in batch order:
            # the DMA engines drain descriptors in post order, so batch 0
            # completes first and the encoder can start early.
            eng = nc.sync
            if b == 0:
                hs_ = [
                    x4p.tile([128, w], f16, tag=f"x4h0{k}", name=f"x4h0{k}")
                    for k, (c0, w) in enumerate(B0REG)
                ]
                ls_ = [
                    x4p.tile([128, w], f16, tag=f"x4l0{k}", name=f"x4l0{k}")
                    for k, (c0, w) in enumerate(B0REG)
                ]
                for k, (c0, w) in enumerate(B0REG):
                    eng.dma_start(out=hs_[k][:, 0:w], in_=xh4[0, :, c0 : c0 + w])
                    eng.dma_start(out=ls_[k][:, 0:w], in_=xl4[0, :, c0 : c0 + w])
                return hs_, ls_
            ht = x4p.tile([128, X4_COLS], f16, tag="x4h", bufs=3, name=f"x4h{b}")
            lt = x4p.tile([128, X4_COLS], f16, tag="x4l", bufs=3, name=f"x4l{b}")
            eng.dma_start(out=ht[:, :], in_=xh4[b])
            eng.dma_start(out=lt[:, :], in_=xl4[b])
            return [ht], [lt]

        def vpost():
            nc.scalar.dma_start(out=vp_sb[:, :], in_=vpack)

        # window tile j -> (x4 region index, local column base) per batch
        def jreg(b, j):
            if b == 0:
                return ((0, 0), (0, 512), (1, 0), (1, 512))[j]
            return (0, 512 * j)

        def emit_enc(b, x4, inserts=()):
            x4h, x4l = x4
            # ---- encoder GEMM + square per window tile -----------------
            h = hp.tile([128, S], f32, tag="h")
            hs[b] = h
            nc.vector.memset(h[:, NWIN:S], 0.0)  # sq j3 reads full 512
            sqs[b] = []
            ins = dict(inserts)
            for j in range(4):
                n, t0 = TN[j], 512 * j
                reg, lc = jreg(b, j)
                ph = php.tile([128, 512], f32, tag="ph")
                terms = []
                for kt in range(KT):
                    terms.append((whi_sb[:, kt, :], x4h[reg], kt))
                for kt in range(KT):
                    terms.append((whi_sb[:, kt, :], x4l[reg], kt))
                for kt in range(KT):
                    terms.append((wlo_sb[:, kt, :], x4h[reg], kt))
                for i, (wt, xs, kt) in enumerate(terms):
                    nc.tensor.matmul(
                        ph[:, 0:n],
                        lhsT=wt,
                        rhs=xs[:, lc + 4 * kt : lc + 4 * kt + n],
                        start=(i == 0),
                        stop=(i == len(terms) - 1),
                    )
                    if (j, i) in ins:
                        ins[(j, i)]()
                nc.vector.tensor_scalar_add(
                    out=h[:, t0 : t0 + n], in0=ph[:, 0:n], scalar1=bc_col
                )
                sq = sqp.tile([128, 512], f32, tag="sq")
                sqs[b].append(sq)
                # full 512 cols (h zero-padded) so the stats rows are fully
                # initialized without a PSUM memset
                nc.vector.tensor_mul(
                    out=sq[:, 0:512], in0=h[:, t0 : t0 + 512],
                    in1=h[:, t0 : t0 + 512],
                )
                if j in ins:
                    ins[j]()

        def emit_stats(b):
            # 4-way column-packed burst of ones-matmul partition reductions.
            # The bank is memset first (cheap) so the full-width NR reads
            # below never see uninitialized PSUM rows; the matmuls then
            # overwrite rows {0,32,64,96}.
            ps = psp.tile([128, 512], f32, tag="ps")
            pss[b] = ps
            nc.vector.memset(ps[:, :], 0.0)
            for j in range(4):
                nc.tensor.matmul(
                    ps[32 * j : 32 * j + 1, 0:512],
                    lhsT=ones_sb[:, 0:1],
                    rhs=sqs[b][j][:, 0:512],
                    start=True,
                    stop=True,
                    tile_position=(0, 32 * j),
                )

        def emit_nr(b):
            ps = pss[b]
            # ---- rstd = (var + eps)**-0.5 via Newton ------------------
            # Engines cannot stride the partition dim, so the 4 real stats
            # rows are bounced through DRAM into a compact [128, 16] tile:
            # vsh[p, 4a+chi] = var(tile a, window col 128*chi + p).  The
            # Newton chain then runs on GPSIMD over 16 columns (all 8 Q7
            # cores, 256 elem each) instead of 512 full-width - ~25x less
            # work, off the DVE FIFO entirely.
            # v = ps/H + eps on the Scalar engine: ACT reads PSUM, Copy is
            # exact (no spline table), and it keeps this latency-critical
            # step out of the DVE FIFO where it queued behind normalize work
            v = nrp.tile([128, 512], f32, tag="v")
            nc.scalar.activation(
                out=v[:, :], in_=ps[:, :], func=AF.Copy,
                bias=LN_EPS, scale=1.0 / H,
            )
            # DMA1: rows {0,32,64,96} -> DRAM as a flat [4, 512] image.
            # Posted on the scalar queue right behind its producer v, so
            # there is no cross-engine semaphore hop before the DMA starts.
            rdv = drp.tile([4, 512], f32, tag="rdv")
            nc.scalar.dma_start(out=rdv[:, :], in_=srows(v))
            # DMA2: read the same 2048 values back as [128, 16] - a pure
            # linear reshape (both sides traverse the buffer in flat order)
            vsh = nrp.tile([128, 16], f32, tag="vsh")
            rdv_flat = bass.AP(
                tensor=rdv.tensor, offset=rdv.offset, ap=[[16, 128], [1, 16]]
            )
            nc.sync.dma_start(out=vsh[:, :], in_=rdv_flat)
            ya = nrp.tile([128, 16], f32, tag="ya")
            yb = nrp.tile([128, 16], f32, tag="yb")
            nc.gpsimd.tensor_scalar(
                out=ya[:, :], in0=vsh[:, :], scalar1=float(_SEED_B),
                scalar2=float(_SEED_A), op0=OP.mult, op1=OP.add,
            )
            ycur, ynxt = ya, yb
            for _ in range(NR_ITERS):
                y2 = nrp.tile([128, 16], f32, tag="y2")
                nc.gpsimd.tensor_mul(out=y2[:, :], in0=ycur[:, :], in1=ycur[:, :])
                nc.gpsimd.tensor_mul(out=y2[:, :], in0=y2[:, :], in1=vsh[:, :])
                nc.gpsimd.tensor_scalar(
                    out=y2[:, :], in0=y2[:, :], scalar1=-0.5, scalar2=1.5,
                    op0=OP.mult, op1=OP.add,
                )
                nc.gpsimd.tensor_mul(
                    out=ynxt[:, :], in0=ycur[:, :], in1=y2[:, :]
                )
                ycur, ynxt = ynxt, ycur
            # rstd -> DRAM (flat) for the partition broadcast read-back
            rd = drp.tile([128, 16], f32, tag="rd")
            rds[b] = rd
            nc.sync.dma_start(out=rd[:, :], in_=ycur[:, :])

        prs = {}

        def emit_bcast(b):
            # partition-broadcast read of the flat rstd buffer, split into
            # 4 per-tile chunks so norm_pre(j) can start as soon as its own
            # chunk lands: pr[p, 512a + c] = rstd[a, c].  Posted early (the
            # ~3us 1MB transfer is the longest latency in the chain).
            rd = rds[b]
            pr = prp.tile([128, S], f32, tag="pr")
            prs[b] = pr
            for a in range(4):
                chunk = bass.AP(
                    tensor=rd.tensor, offset=rd.offset + 512 * a,
                    ap=[[0, 128], [1, 512]],
                )
                nc.sync.dma_start(out=pr[:, 512 * a : 512 * a + 512], in_=chunk)

        def emit_premul(b):
            # normalize multiplies, emitted a full batch AFTER the bcast
            # post: by their queue position the pr data has already landed,
            # so they are promptly-ready and can never head-of-line-block
            # the DVE FIFO (the scheduler kept hoisting bcast-gated mults
            # in front of squares the PE was about to need).
            h = hs[b]
            pr = prs[b]
            pre = prep.tile([128, S], f32, tag="pre")
            pres_all[b] = pre
            for j in range(4):
                n, t0 = TN[j], 512 * j
                nc.vector.tensor_mul(
                    out=pre[:, t0 : t0 + n], in0=h[:, t0 : t0 + n],
                    in1=pr[:, t0 : t0 + n],
                )

        def emit_norm_fin(b):
            # gelu -> enc f32 (ACT only; the f16 hi/lo pair is derived by
            # DVE cast+sub later, scheduled where it can't block anything
            # the PE is about to need)
            pre = pres_all[b]
            encf = encfp.tile([128, S], f32, tag="encf")
            encfs_all[b] = encf
            for j in range(4):
                n, t0 = TN[j], 512 * j
                # gamma rides the ACT per-partition scale; beta the bias
                nc.scalar.activation(
                    out=encf[:, t0 : t0 + n], in_=pre[:, t0 : t0 + n],
                    func=AF.Gelu, bias=beta_col, scale=gamma_col,
                )

        def emit_subs(b, tiles=(0, 1, 2, 3)):
            # enc f16 pair: ench = f16(encf); encl = f16(encf - ench)
            encf = encfs_all[b]
            if b not in enchs:
                enchs[b] = enchp.tile([128, S], f16, tag="ench", name="ench")
                encls[b] = enclp.tile([128, S], f16, tag="encl", name="encl")
            ench, encl = enchs[b], encls[b]
            for j in tiles:
                n, t0 = TN[j], 512 * j
                nc.vector.tensor_copy(
                    out=ench[:, t0 : t0 + n], in_=encf[:, t0 : t0 + n]
                )
                nc.vector.tensor_sub(
                    out=encl[:, t0 : t0 + n], in0=encf[:, t0 : t0 + n],
                    in1=ench[:, t0 : t0 + n],
                )

        def emit_cmp(b, do_subs=True, after_l1j0=None):
            # ---- comparator MLP ----------------------------------------
            # PE order L1j0 L1j1 L1j2 L2A L1j3 L3j0 L3j1 L2B L3j2 L3j3:
            # every packed fp32 matmul's operands are ready >1 L1-group
            # before it issues, so the col/row-group concurrency is never
            # dependency-serialized.
            if do_subs:
                emit_subs(b)
            ench, encl = enchs[b], encls[b]
            pz3 = pz3p.tile([128, 512], f32, tag="pz3")
            nc.vector.memset(pz3[:, :], 0.0)

            z1s = [None] * 4

            def l1(j):
                n, t0 = CN[j], 512 * j
                pz1 = pz1p.tile([128, 512], f32, tag="pz1")
                terms = [
                    (w1h_sb[:, 0, :], ench, t0),
                    (w1h_sb[:, 1, :], ench, t0 + W),
                    (w1l_sb[:, 0, :], ench, t0),
                    (w1l_sb[:, 1, :], ench, t0 + W),
                    (w1h_sb[:, 0, :], encl, t0),
                    (w1h_sb[:, 1, :], encl, t0 + W),
                ]
                for i, (wt, e, c0) in enumerate(terms):
                    nc.tensor.matmul(
                        pz1[:, 0:n], lhsT=wt, rhs=e[:, c0 : c0 + n],
                        start=(i == 0), stop=(i == len(terms) - 1),
                    )
                if j == 0 and after_l1j0 is not None:
                    after_l1j0()
                z1 = z1p.tile([128, 512], f32, tag="z1")
                z1s[j] = z1
                nc.scalar.activation(
                    out=z1[:, 0:n], in_=pz1[:, 0:n], func=AF.Gelu,
                    bias=b1_col, scale=1.0,
                )

            def l2(pair):
                # 2-way column-packed pair into one PSUM bank
                pz2 = pz2p.tile([128, 512], f32, tag="pz2")
                if pair[1] == 3:
                    nc.vector.memset(pz2[64:128, CN[3] : 512], 0.0)
                for k, j in enumerate(pair):
                    nc.tensor.matmul(
                        pz2[64 * k : 64 * k + 64, 0 : CN[j]],
                        lhsT=w2_sb[:, :], rhs=z1s[j][:, 0 : CN[j]],
                        start=True, stop=True,
                        tile_position=(0, 64 * k),
                    )
                z2 = z2p.tile([128, 512], f32, tag="z2")
                nc.scalar.activation(
                    out=z2[:, :], in_=pz2[:, :], func=AF.Gelu,
                    bias=b2_col, scale=1.0,
                )
                return z2

            def l3(j, z2):
                # row+column-packed quad member into the shared pz3 bank
                r0 = 64 * (j % 2)
                nc.tensor.matmul(
                    pz3[32 * j : 32 * j + 1, 0 : CN[j]],
                    lhsT=w3_sb[r0 : r0 + 64, 0:1],
                    rhs=z2[r0 : r0 + 64, 0 : CN[j]],
                    start=True, stop=True,
                    tile_position=(r0, 32 * j),
                )

            l1(0)
            l1(1)
            l1(2)
            z2a = l2((0, 1))
            l1(3)
            l3(0, z2a)
            l3(1, z2a)
            z2b = l2((2, 3))
            l3(2, z2b)
            l3(3, z2b)
            # PSUM -> SBUF logits copy rides the Scalar engine (ACT Copy)
            # so the wait-on-L3 doesn't block the DVE queue
            lg = lgp.tile([128, 512], f32, tag="lg")
            nc.scalar.activation(out=lg[:, :], in_=pz3[:, :], func=AF.Copy)
            # out-DMA posts on the scalar queue, right behind its producer,
            # keeping the sync queue free for the rstd chain DMAs
            nc.scalar.dma_start(out=out[b], in_=srows(lg))

        # ---- software-pipelined emission -------------------------------
        # PE stream: enc0, enc1[stats0 mid-j0], enc2[stats1], enc3
        # [stats2], cmp0[stats3 after L1j0], cmp1..cmp3.  Each stats burst
        # is injected a few matmuls into the NEXT batch's first window
        # tile (just enough cover for the last sq to land), so the rstd
        # chain starts as early as possible; the chain then drains on
        # gpsimd/sync while the PE streams on.  norm_fin gelus are emitted
        # as encoder inserts so the ACT queue order is [v(b), gelus(b-1)]
        # and the latency-critical v never queues behind stale gelus.
        # subs(0) is split around enc3's j-tiles so enc3's last drains
        # (which gate stats(3)) aren't pushed behind 4 tiles of cast/sub.
        def stats_nr(b):
            return lambda: (emit_stats(b), emit_nr(b))

        def nfin(b):
            return lambda: emit_norm_fin(b)

        def subs_t(b, tiles):
            return lambda: emit_subs(b, tiles)

        def premul(b):
            return lambda: emit_premul(b)

        vpost()
        x4s = [emit_x4(b) for b in range(NB)]
        emit_enc(0, x4s[0])
        emit_enc(1, x4s[1], inserts={(0, 7): stats_nr(0)})
        emit_bcast(0)
        emit_enc(
            2, x4s[2],
            inserts={(0, 3): premul(0), (0, 7): stats_nr(1), 0: nfin(0)},
        )
        emit_bcast(1)
        emit_enc(
            3, x4s[3],
            inserts={
                (0, 3): premul(1),
                (0, 7): stats_nr(2),
                0: nfin(1),
                1: subs_t(0, (0, 1)),
                2: subs_t(0, (2, 3)),
            },
        )
        emit_bcast(2)
        emit_cmp(
            0, do_subs=False,
            after_l1j0=lambda: (emit_premul(2), emit_stats(3), emit_nr(3)),
        )
        emit_subs(1)
        emit_norm_fin(2)
        emit_cmp(1, do_subs=False)
        emit_bcast(3)
        emit_subs(2)
        emit_premul(3)
        emit_norm_fin(3)
        emit_cmp(2, do_subs=False)
        emit_subs(3)
        emit_cmp(3, do_subs=False)

    nc.compile()
    return nc


def _get_nc():
    if "nc" not in _BUILT:
        _BUILT["nc"] = _build_nc()
    return _BUILT["nc"]


def make_in_maps(x, W_enc, b_enc, gamma, beta, W1, b1, W2, b2, W3, b3):
    """Host-side prep: shard x, build the 4-shift X4 stacks, center the
    encoder weights, pack weights/vectors into two DMA-able blobs."""
    x = np.ascontiguousarray(np.asarray(x, np.float32))
    W_enc = np.asarray(W_enc, np.float32)
    b_enc = np.asarray(b_enc, np.float32)

    W_c = W_enc - W_enc.mean(axis=1, keepdims=True)
    b_c = b_enc - b_enc.mean()
    wct = W_c.reshape(KT, 128, 128).transpose(1, 0, 2)
    wench = wct.astype(np.float16)
    wencl = (wct - wench.astype(np.float32)).astype(np.float16)
    w1t = np.asarray(W1, np.float32).reshape(2, 128, 128).transpose(1, 0, 2)
    w1h = w1t.astype(np.float16)
    w1l = (w1t - w1h.astype(np.float32)).astype(np.float16)
    wpack = np.ascontiguousarray(
        np.concatenate([wench, wencl, w1h, w1l], axis=1)
    )  # [128, 14, 128] f16

    vpack = np.zeros((128, 73), np.float32)
    vpack[:, 0:64] = np.asarray(W2, np.float32)
    w3c = np.asarray(W3, np.float32).reshape(64)
    vpack[0:64, 64] = w3c
    vpack[64:128, 64] = w3c
    vpack[:, 65] = b_c
    vpack[:, 66] = np.asarray(gamma, np.float32)
    vpack[:, 67] = np.asarray(beta, np.float32)
    vpack[:, 68] = np.asarray(b1, np.float32)
    b2f = np.asarray(b2, np.float32)
    vpack[0:64, 69] = b2f
    vpack[64:128, 69] = b2f

    xT = x.transpose(0, 2, 1)  # [B, 32, S]
    xTh = xT.astype(np.float16)
    xTl = (xT - xTh.astype(np.float32)).astype(np.float16)
    # X4[b, 32j+i, s] = xT[b, i, j+s]
    x4h = np.empty((B, 128, X4_COLS), np.float16)
    x4l = np.empty((B, 128, X4_COLS), np.float16)
    for j in range(4):
        x4h[:, 32 * j : 32 * j + 32, :] = xTh[:, :, j : j + X4_COLS]
        x4l[:, 32 * j : 32 * j + 32, :] = xTl[:, :, j : j + X4_COLS]

    in_maps = []
    for c in range(NCORES):
        sl = slice(NB * c, NB * (c + 1))
        in_maps.append(
            dict(
                xh4=np.ascontiguousarray(x4h[sl]),
                xl4=np.ascontiguousarray(x4l[sl]),
                wpack=wpack,
                vpack=vpack,
            )
        )
    return in_maps


def assemble_output(core_outs, b3):
    """core_outs: list of 8 arrays [NB, 4, 512] of pre-b3 logits."""
    b3 = float(np.asarray(b3).reshape(-1)[0])
    logits = np.zeros((B, T), np.float32)
    for c, o in enumerate(core_outs):
        for bb in range(NB):
            row = []
            for j in range(4):
                row.append(o[bb, j, 0 : CN[j]])
            logits[NB * c + bb] = np.concatenate(row)
    z = (logits + b3).astype(np.float32)
    p = (1.0 / (1.0 + np.exp(-z.astype(np.float64)))).astype(np.float32)
    probs = np.zeros((B, S), np.float32)
    probs[:, W : W + T] = p
    return probs, probs > 0.5


def kernel(**inputs):
    from concourse.bass_utils import run_bass_kernel_spmd

    nc = _get_nc()
    in_maps = make_in_maps(**inputs)
    res = run_bass_kernel_spmd(nc, in_maps, core_ids=list(range(NCORES)))
    core_outs = [res.results[c]["out"] for c in range(NCORES)]
    return assemble_output(core_outs, inputs["b3"])
